# revision 2
# baseline (speedup 1.0000x reference)
"""Trainium2 Bass kernel for nn_DecoderLSTM (ragged LSTM decoder + vocab projection).

Strategy (8 NeuronCores, SPMD):
  - Host: stable-sort batch by descending caption length, gather embeddings for
    the *active* (b, t) pairs only (lengths sorted desc => active set at step t
    is a prefix of the batch), pre-transpose/pack all matmul operands, cast to
    bf16.
  - Device (identical program on all 8 cores; per-core data = lin_W vocab
    shard of 4000 rows):
      * LSTM recurrence over Td steps, full batch replicated on every core.
        gates = [x; h] @ [W_ih; W_hh].T as 8 accumulating K=128 matmuls with
        the (bf16) weights as the moving operand, into one PSUM region.
      * h kept in transposed packed layout HT [128, 4, 64+NA] (bf16) via PE
        transposes, so it directly serves as the stationary operand of both
        the next-step gates matmul and the prediction matmul.
      * Prediction matmul over packed active rows in M=128 chunks (full PE
        stationary utilization), N = 4000 local vocab, interleaved with the
        recurrence as h columns become available.
      * Inactive (b, t) outputs zero-filled by DMA from a zero tile.
  - Host: concatenate the 8 vocab shards.

kernel() accepts the full unsharded inputs and returns
(predictions, caps_sorted, decode_lengths, sort_ind) exactly like reference().
"""

import math

import numpy as np
import ml_dtypes

import concourse.bass as bass
import concourse.mybir as mybir
import concourse.tile as tile
from concourse import bacc
from concourse.bass_utils import run_bass_kernel_spmd
from concourse.masks import make_identity

B, T, E, H, V = 64, 52, 512, 512, 32000
NCORES = 8
VL = V // NCORES          # 4000 local vocab rows per core
TDF = T - 1               # 51 output timesteps
G = 4 * H                 # 2048 gate columns
KE = E // 128             # 4 K-chunks for the x side
KH = H // 128             # 4 K-chunks for the h side
NT = 512                  # matmul moving-operand chunk (one PSUM bank of fp32)
PN = 500                  # prediction matmul N-chunk (4000 = 8 * 500)

F32 = mybir.dt.float32
BF16 = mybir.dt.bfloat16
AF = mybir.ActivationFunctionType

# test harness hooks
TRACE = False
TRACE_KWARGS = {}
LAST_RESULTS = None


def _build(nc, Td, bs, off, na, has_linb):
    """Emit the Tile program. bs[t] = #active batch items at step t (prefixes,
    non-increasing); off[t] = sum(bs[:t]); na = off[Td] total active pairs."""
    xt_d = nc.dram_tensor("xt", [128, KE, na], BF16, kind="ExternalInput")
    ht0_d = nc.dram_tensor("ht0", [128, KH, B], BF16, kind="ExternalInput")
    wx_d = nc.dram_tensor("wx", [128, KE, G], BF16, kind="ExternalInput")
    wh_d = nc.dram_tensor("wh", [128, KH, G], BF16, kind="ExternalInput")
    lw_d = nc.dram_tensor("lw", [128, KH, VL], BF16, kind="ExternalInput")
    bias_d = nc.dram_tensor("bias", [B, G], F32, kind="ExternalInput")
    if has_linb:
        linb_d = nc.dram_tensor("linb", [128, VL], F32, kind="ExternalInput")
    out_d = nc.dram_tensor("out", [B, TDF, VL], F32, kind="ExternalOutput")

    n_chunks = math.ceil(na / 128)

    with tile.TileContext(nc) as tc:
        with (
            tc.tile_pool(name="const", bufs=1) as const,
            tc.tile_pool(name="gpsum", bufs=1, space="PSUM") as gpsum,
            tc.tile_pool(name="ppsum", bufs=2, space="PSUM") as ppsum,
            tc.tile_pool(name="tpsum", bufs=2, space="PSUM") as tpsum,
            tc.tile_pool(name="work", bufs=2) as work,
            tc.tile_pool(name="pwork", bufs=2) as pwork,
        ):
            WX = const.tile([128, KE, G], BF16, tag="WX")
            nc.sync.dma_start(WX[:], wx_d[:, :, :])
            WH = const.tile([128, KH, G], BF16, tag="WH")
            nc.sync.dma_start(WH[:], wh_d[:, :, :])
            LW = const.tile([128, KH, VL], BF16, tag="LW")
            nc.sync.dma_start(LW[:], lw_d[:, :, :])
            XT = const.tile([128, KE, na], BF16, tag="XT")
            nc.sync.dma_start(XT[:], xt_d[:, :, :])
            BIAS = const.tile([B, G], F32, tag="BIAS")
            nc.sync.dma_start(BIAS[:], bias_d[:, :])
            # h-transposed history: cols 0:B = h0 (image features), block t at
            # B+off[t] (width bs[t])
            HT = const.tile([128, KH, B + na], BF16, tag="HT")
            nc.sync.dma_start(HT[:, :, 0:B], ht0_d[:, :, :])
            if has_linb:
                LB = const.tile([128, VL], F32, tag="LB")
                nc.sync.dma_start(LB[:], linb_d[:, :])
            CST = const.tile([B, H], F32, tag="CST")
            nc.vector.memset(CST[:], 0.0)
            ZERO = const.tile([B, VL], F32, tag="ZERO")
            nc.vector.memset(ZERO[:], 0.0)
            IDN = const.tile([B, B], BF16, tag="IDN")
            make_identity(nc, IDN[:])

            # zero-fill the inactive region of the output (independent of
            # compute; the scheduler can run these DMAs any time)
            for t in range(TDF):
                bt = bs[t] if t < Td else 0
                if bt < B:
                    nc.sync.dma_start(out_d[bt:B, t, :], ZERO[: B - bt, :])

            def emit_pred_chunk(m):
                base = m * 128
                mw = min(128, na - base)
                pred = pwork.tile([128, VL], F32, tag="pred")
                for n in range(VL // PN):
                    pp = ppsum.tile([128, PN], F32, tag="pp")
                    for k in range(KH):
                        nc.tensor.matmul(
                            pp[:mw, :],
                            HT[:, k, B + base : B + base + mw],
                            LW[:, k, n * PN : (n + 1) * PN],
                            start=(k == 0),
                            stop=(k == KH - 1),
                        )
                    if has_linb:
                        nc.vector.tensor_add(
                            pred[:mw, n * PN : (n + 1) * PN], pp[:mw, :],
                            LB[:mw, n * PN : (n + 1) * PN],
                        )
                    else:
                        nc.vector.tensor_copy(
                            pred[:mw, n * PN : (n + 1) * PN], pp[:mw, :]
                        )
                # scatter rows to DRAM grouped by timestep
                s = int(np.searchsorted(off, base, side="right")) - 1
                g0 = base
                while g0 < base + mw:
                    g1 = min(base + mw, int(off[s + 1]))
                    b0 = g0 - int(off[s])
                    b1 = g1 - int(off[s])
                    nc.sync.dma_start(
                        out_d[b0:b1, s, :], pred[g0 - base : g1 - base, :]
                    )
                    g0 = g1
                    s += 1

            pred_done = 0
            for t in range(Td):
                bt = bs[t]
                hcol = 0 if t == 0 else B + int(off[t - 1])
                xcol = int(off[t])
                ps = gpsum.tile([B, G], F32, tag="gpsum")
                for k in range(KE):
                    for n in range(G // NT):
                        nc.tensor.matmul(
                            ps[:bt, n * NT : (n + 1) * NT],
                            XT[:, k, xcol : xcol + bt],
                            WX[:, k, n * NT : (n + 1) * NT],
                            start=(k == 0),
                            stop=False,
                        )
                for k in range(KH):
                    for n in range(G // NT):
                        nc.tensor.matmul(
                            ps[:bt, n * NT : (n + 1) * NT],
                            HT[:, k, hcol : hcol + bt],
                            WH[:, k, n * NT : (n + 1) * NT],
                            start=False,
                            stop=(k == KH - 1),
                        )
                gates = work.tile([B, G], F32, tag="gates")
                nc.vector.tensor_add(gates[:bt, :], ps[:bt, :], BIAS[:bt, :])
                # gate layout (host-permuted): [i | f | o | g]
                sig = work.tile([B, 3 * H], F32, tag="sig")
                nc.scalar.activation(sig[:bt, :], gates[:bt, 0 : 3 * H], AF.Sigmoid)
                gt = work.tile([B, H], F32, tag="gt")
                nc.scalar.activation(gt[:bt, :], gates[:bt, 3 * H : 4 * H], AF.Tanh)
                ig = work.tile([B, H], F32, tag="ig")
                nc.vector.tensor_mul(ig[:bt, :], sig[:bt, 0:H], gt[:bt, :])
                nc.vector.tensor_mul(CST[:bt, :], sig[:bt, H : 2 * H], CST[:bt, :])
                nc.vector.tensor_add(CST[:bt, :], CST[:bt, :], ig[:bt, :])
                tch = work.tile([B, H], F32, tag="tch")
                nc.scalar.activation(tch[:bt, :], CST[:bt, :], AF.Tanh)
                hbf = work.tile([B, H], BF16, tag="hbf")
                nc.vector.tensor_mul(hbf[:bt, :], sig[:bt, 2 * H : 3 * H], tch[:bt, :])
                ocol = B + int(off[t])
                for k in range(KH):
                    tp = tpsum.tile([128, B], BF16, tag="tp")
                    nc.tensor.transpose(
                        tp[:, :bt], hbf[:bt, k * 128 : (k + 1) * 128], IDN[:bt, :bt]
                    )
                    nc.scalar.copy(HT[:, k, ocol : ocol + bt], tp[:, :bt])
                avail = int(off[t]) + bt
                while pred_done < n_chunks and (pred_done + 1) * 128 <= avail:
                    emit_pred_chunk(pred_done)
                    pred_done += 1
            while pred_done < n_chunks:
                emit_pred_chunk(pred_done)
                pred_done += 1
    return nc


def _kpack(m):
    """[D, C] (D % 128 == 0) -> [128, D//128, C] K-chunked layout."""
    d, c = m.shape
    return np.ascontiguousarray(
        m.reshape(d // 128, 128, c).transpose(1, 0, 2)
    )


def kernel(image_features, encoded_captions, caption_lengths,
           emb_W, W_ih, W_hh, b_ih, b_hh, lin_W, lin_b):
    global LAST_RESULTS
    bf = ml_dtypes.bfloat16
    img = np.asarray(image_features, np.float32)
    caps_in = np.asarray(encoded_captions)
    cl = np.asarray(caption_lengths)
    embW = np.asarray(emb_W, np.float32)
    Wih = np.asarray(W_ih, np.float32)
    Whh = np.asarray(W_hh, np.float32)
    bih = np.asarray(b_ih, np.float32)
    bhh = np.asarray(b_hh, np.float32)
    linW = np.asarray(lin_W, np.float32)
    linb = np.asarray(lin_b, np.float32)

    lengths = cl[:, 0]
    sort_ind = np.argsort(-lengths, kind="stable")
    caps = caps_in[sort_ind]
    dl = lengths[sort_ind] - 1
    img_s = img[sort_ind]

    Td = int(dl.max())
    bs = [int((dl > t).sum()) for t in range(Td)]
    off = np.concatenate([[0], np.cumsum(bs)]).astype(np.int64)
    na = int(off[Td])

    xp = np.empty((na, E), np.float32)
    for t in range(Td):
        xp[off[t]:off[t + 1]] = embW[caps[: bs[t], t]]

    # gate order [i, f, o, g] (torch rows are i, f, g, o)
    perm = np.concatenate([
        np.arange(0, H), np.arange(H, 2 * H),
        np.arange(3 * H, 4 * H), np.arange(2 * H, 3 * H),
    ])
    xt = _kpack(np.ascontiguousarray(xp.T).astype(bf))
    ht0 = _kpack(np.ascontiguousarray(img_s.T).astype(bf))
    wx = _kpack(np.ascontiguousarray(Wih[perm].T).astype(bf))
    wh = _kpack(np.ascontiguousarray(Whh[perm].T).astype(bf))
    bias = np.ascontiguousarray(
        np.broadcast_to((bih + bhh)[perm].astype(np.float32), (B, G))
    )
    has_linb = bool(np.any(linb != 0.0))

    nc = bacc.Bacc("TRN2", target_bir_lowering=False, debug=False,
                   num_devices=NCORES)
    _build(nc, Td, bs, off, na, has_linb)
    nc.finalize()

    in_maps = []
    for c in range(NCORES):
        lw = _kpack(np.ascontiguousarray(linW[c * VL:(c + 1) * VL].T).astype(bf))
        m = dict(xt=xt, ht0=ht0, wx=wx, wh=wh, lw=lw, bias=bias)
        if has_linb:
            m["linb"] = np.ascontiguousarray(
                np.broadcast_to(linb[c * VL:(c + 1) * VL].astype(np.float32),
                                (128, VL)))
        in_maps.append(m)

    res = run_bass_kernel_spmd(
        nc, in_maps, core_ids=list(range(NCORES)),
        trace=TRACE, trace_cores=list(range(NCORES)) if TRACE else None,
        **TRACE_KWARGS,
    )
    LAST_RESULTS = res

    preds = np.zeros((B, TDF, V), np.float32)
    for c in range(NCORES):
        preds[:, :, c * VL:(c + 1) * VL] = res.results[c]["out"]
    return preds, caps, dl, sort_ind


# revision 9
# speedup vs baseline: 1.1156x; 1.1156x over previous
"""Trainium2 Bass kernel for nn_DecoderLSTM (ragged LSTM decoder + vocab projection).

Strategy (8 NeuronCores, SPMD):
  - Host: stable-sort batch by descending caption length, gather embeddings for
    the *active* (b, t) pairs only (lengths sorted desc => active set at step t
    is a prefix of the batch), pre-transpose/pack all matmul operands, cast to
    bf16.
  - Device (identical program on all 8 cores; per-core data = lin_W vocab
    shard of 4000 rows):
      * LSTM recurrence over Td steps, full batch replicated on every core.
        gates = [x; h] @ [W_ih; W_hh].T as 8 accumulating K=128 matmuls with
        the (bf16) weights as the moving operand, into one PSUM region.
      * h kept in transposed packed layout HT [128, 4, 64+NA] (bf16) via PE
        transposes, so it directly serves as the stationary operand of both
        the next-step gates matmul and the prediction matmul.
      * Prediction matmul over packed active rows in M=128 chunks (full PE
        stationary utilization), N = 4000 local vocab, interleaved with the
        recurrence as h columns become available.
      * Inactive (b, t) outputs zero-filled by DMA from a zero tile.
  - Host: concatenate the 8 vocab shards.

kernel() accepts the full unsharded inputs and returns
(predictions, caps_sorted, decode_lengths, sort_ind) exactly like reference().
"""

import math

import numpy as np
import ml_dtypes

import concourse.bass as bass
import concourse.mybir as mybir
import concourse.tile as tile
from concourse import bacc
from concourse.bass_utils import run_bass_kernel_spmd
from concourse.masks import make_identity

B, T, E, H, V = 64, 52, 512, 512, 32000
NCORES = 8
VL = V // NCORES          # 4000 local vocab rows per core
TDF = T - 1               # 51 output timesteps
G = 4 * H                 # 2048 gate columns
KE = E // 128             # 4 K-chunks for the x side
KH = H // 128             # 4 K-chunks for the h side
NT = 512                  # matmul moving-operand chunk (one PSUM bank of fp32)
PN = 500                  # prediction matmul N-chunk (4000 = 8 * 500)

F32 = mybir.dt.float32
BF16 = mybir.dt.bfloat16
AF = mybir.ActivationFunctionType

# test harness hooks
TRACE = False
TRACE_KWARGS = {}
LAST_RESULTS = None


def _build(nc, Td, bs, off, na, has_linb):
    """Emit the Tile program. bs[t] = #active batch items at step t (prefixes,
    non-increasing); off[t] = sum(bs[:t]); na = off[Td] total active pairs."""
    xt_d = nc.dram_tensor("xt", [128, KE, na], BF16, kind="ExternalInput")
    ht0_d = nc.dram_tensor("ht0", [128, KH, B], BF16, kind="ExternalInput")
    wx_d = nc.dram_tensor("wx", [128, KE, G], BF16, kind="ExternalInput")
    wh_d = nc.dram_tensor("wh", [128, KH, G], BF16, kind="ExternalInput")
    lw_d = nc.dram_tensor("lw", [128, KH, VL], BF16, kind="ExternalInput")
    bias_d = nc.dram_tensor("bias", [B, G], F32, kind="ExternalInput")
    if has_linb:
        linb_d = nc.dram_tensor("linb", [128, VL], F32, kind="ExternalInput")
    # t-major so each step's rows are contiguous in DRAM (big DMA descriptors)
    out_d = nc.dram_tensor("out", [TDF, B, VL], F32, kind="ExternalOutput")

    n_chunks = math.ceil(na / 128)

    with tile.TileContext(nc) as tc:
        with (
            tc.tile_pool(name="const", bufs=1) as const,
            tc.tile_pool(name="gpsum", bufs=4, space="PSUM") as gpsum,
            tc.tile_pool(name="ppsum", bufs=2, space="PSUM") as ppsum,
            tc.tile_pool(name="tpsum", bufs=2, space="PSUM") as tpsum,
            tc.tile_pool(name="work", bufs=2) as work,
            tc.tile_pool(name="pwork", bufs=3) as pwork,
        ):
            WX = const.tile([128, KE, G], BF16, tag="WX")
            nc.sync.dma_start(WX[:], wx_d[:, :, :])
            WH = const.tile([128, KH, G], BF16, tag="WH")
            nc.sync.dma_start(WH[:], wh_d[:, :, :])
            LW = const.tile([128, KH, VL], BF16, tag="LW")
            nc.sync.dma_start(LW[:], lw_d[:, :, :])
            XT = const.tile([128, KE, na], BF16, tag="XT")
            nc.sync.dma_start(XT[:], xt_d[:, :, :])
            BIAS = const.tile([B, G], F32, tag="BIAS")
            nc.sync.dma_start(BIAS[:], bias_d[:, :])
            # h-transposed history: cols 0:B = h0 (image features), block t at
            # B+off[t] (width bs[t])
            HT = const.tile([128, KH, B + na], BF16, tag="HT")
            nc.sync.dma_start(HT[:, :, 0:B], ht0_d[:, :, :])
            if has_linb:
                LB = const.tile([128, VL], F32, tag="LB")
                nc.sync.dma_start(LB[:], linb_d[:, :])
            CST = const.tile([B, H], F32, tag="CST")
            nc.vector.memset(CST[:], 0.0)
            ZERO = const.tile([B, VL], F32, tag="ZERO")
            nc.vector.memset(ZERO[:], 0.0)
            IDN = const.tile([B, B], BF16, tag="IDN")
            make_identity(nc, IDN[:])

            # zero-fill the inactive region of the output (independent of
            # compute; the scheduler can run these DMAs any time). t-major
            # layout makes each fill one contiguous descriptor.
            for t in range(TDF):
                bt = bs[t] if t < Td else 0
                if bt < B:
                    nc.sync.dma_start(out_d[t, bt:B, :], ZERO[: B - bt, :])

            def emit_pred_chunk(m):
                base = m * 128
                mw = min(128, na - base)
                pred = pwork.tile([128, VL], F32, tag="pred")
                for n in range(VL // PN):
                    pp = ppsum.tile([128, PN], F32, tag="pp")
                    for k in range(KH):
                        nc.tensor.matmul(
                            pp[:mw, :],
                            HT[:, k, B + base : B + base + mw],
                            LW[:, k, n * PN : (n + 1) * PN],
                            start=(k == 0),
                            stop=(k == KH - 1),
                        )
                    if has_linb:
                        nc.vector.tensor_add(
                            pred[:mw, n * PN : (n + 1) * PN], pp[:mw, :],
                            LB[:mw, n * PN : (n + 1) * PN],
                        )
                    else:
                        nc.vector.tensor_copy(
                            pred[:mw, n * PN : (n + 1) * PN], pp[:mw, :]
                        )
                # scatter rows to DRAM grouped by timestep; each group is one
                # contiguous DRAM range in the t-major layout
                s = int(np.searchsorted(off, base, side="right")) - 1
                g0 = base
                while g0 < base + mw:
                    g1 = min(base + mw, int(off[s + 1]))
                    b0 = g0 - int(off[s])
                    b1 = g1 - int(off[s])
                    nc.sync.dma_start(
                        out_d[s, b0:b1, :], pred[g0 - base : g1 - base, :]
                    )
                    g0 = g1
                    s += 1

            pred_done = 0
            for t in range(Td):
                bt = bs[t]
                hcol = 0 if t == 0 else B + int(off[t - 1])
                xcol = int(off[t])
                # gate layout (host-permuted): [g | i | f | o]; per-512-column
                # PSUM tiles so elementwise pipelines with the matmul stream
                acts = []
                for n in range(G // NT):
                    ps = gpsum.tile([B, NT], F32, tag="gps")
                    for k in range(KE):
                        nc.tensor.matmul(
                            ps[:bt, :],
                            XT[:, k, xcol : xcol + bt],
                            WX[:, k, n * NT : (n + 1) * NT],
                            start=(k == 0),
                            stop=False,
                        )
                    for k in range(KH):
                        nc.tensor.matmul(
                            ps[:bt, :],
                            HT[:, k, hcol : hcol + bt],
                            WH[:, k, n * NT : (n + 1) * NT],
                            start=False,
                            stop=(k == KH - 1),
                        )
                    pre = work.tile([B, NT], F32, tag=f"pre{n}")
                    nc.vector.tensor_add(
                        pre[:bt, :], ps[:bt, :], BIAS[:bt, n * NT : (n + 1) * NT]
                    )
                    av = work.tile([B, NT], F32, tag=f"act{n}")
                    nc.scalar.activation(
                        av[:bt, :], pre[:bt, :], AF.Tanh if n == 0 else AF.Sigmoid
                    )
                    acts.append(av)
                gv, iv, fv, ov = acts
                ig = work.tile([B, H], F32, tag="ig")
                nc.vector.tensor_mul(ig[:bt, :], iv[:bt, :], gv[:bt, :])
                nc.vector.tensor_mul(CST[:bt, :], fv[:bt, :], CST[:bt, :])
                nc.vector.tensor_add(CST[:bt, :], CST[:bt, :], ig[:bt, :])
                tch = work.tile([B, H], F32, tag="tch")
                nc.scalar.activation(tch[:bt, :], CST[:bt, :], AF.Tanh)
                hbf = work.tile([B, H], BF16, tag="hbf")
                nc.vector.tensor_mul(hbf[:bt, :], ov[:bt, :], tch[:bt, :])
                ocol = B + int(off[t])
                for k in range(KH):
                    tp = tpsum.tile([128, B], BF16, tag="tp")
                    nc.tensor.transpose(
                        tp[:, :bt], hbf[:bt, k * 128 : (k + 1) * 128], IDN[:bt, :bt]
                    )
                    nc.scalar.copy(HT[:, k, ocol : ocol + bt], tp[:, :bt])
                avail = int(off[t]) + bt
                while pred_done < n_chunks and (pred_done + 1) * 128 <= avail:
                    emit_pred_chunk(pred_done)
                    pred_done += 1
            while pred_done < n_chunks:
                emit_pred_chunk(pred_done)
                pred_done += 1
    return nc


def _kpack(m):
    """[D, C] (D % 128 == 0) -> [128, D//128, C] K-chunked layout."""
    d, c = m.shape
    return np.ascontiguousarray(
        m.reshape(d // 128, 128, c).transpose(1, 0, 2)
    )


def kernel(image_features, encoded_captions, caption_lengths,
           emb_W, W_ih, W_hh, b_ih, b_hh, lin_W, lin_b):
    global LAST_RESULTS
    bf = ml_dtypes.bfloat16
    img = np.asarray(image_features, np.float32)
    caps_in = np.asarray(encoded_captions)
    cl = np.asarray(caption_lengths)
    embW = np.asarray(emb_W, np.float32)
    Wih = np.asarray(W_ih, np.float32)
    Whh = np.asarray(W_hh, np.float32)
    bih = np.asarray(b_ih, np.float32)
    bhh = np.asarray(b_hh, np.float32)
    linW = np.asarray(lin_W, np.float32)
    linb = np.asarray(lin_b, np.float32)

    lengths = cl[:, 0]
    sort_ind = np.argsort(-lengths, kind="stable")
    caps = caps_in[sort_ind]
    dl = lengths[sort_ind] - 1
    img_s = img[sort_ind]

    Td = int(dl.max())
    bs = [int((dl > t).sum()) for t in range(Td)]
    off = np.concatenate([[0], np.cumsum(bs)]).astype(np.int64)
    na = int(off[Td])

    xp = np.empty((na, E), np.float32)
    for t in range(Td):
        xp[off[t]:off[t + 1]] = embW[caps[: bs[t], t]]

    # gate order [g, i, f, o] (torch rows are i, f, g, o)
    perm = np.concatenate([
        np.arange(2 * H, 3 * H), np.arange(0, H),
        np.arange(H, 2 * H), np.arange(3 * H, 4 * H),
    ])
    xt = _kpack(np.ascontiguousarray(xp.T).astype(bf))
    ht0 = _kpack(np.ascontiguousarray(img_s.T).astype(bf))
    wx = _kpack(np.ascontiguousarray(Wih[perm].T).astype(bf))
    wh = _kpack(np.ascontiguousarray(Whh[perm].T).astype(bf))
    bias = np.ascontiguousarray(
        np.broadcast_to((bih + bhh)[perm].astype(np.float32), (B, G))
    )
    has_linb = bool(np.any(linb != 0.0))

    nc = bacc.Bacc("TRN2", target_bir_lowering=False, debug=False,
                   num_devices=NCORES)
    _build(nc, Td, bs, off, na, has_linb)
    nc.finalize()

    in_maps = []
    for c in range(NCORES):
        lw = _kpack(np.ascontiguousarray(linW[c * VL:(c + 1) * VL].T).astype(bf))
        m = dict(xt=xt, ht0=ht0, wx=wx, wh=wh, lw=lw, bias=bias)
        if has_linb:
            m["linb"] = np.ascontiguousarray(
                np.broadcast_to(linb[c * VL:(c + 1) * VL].astype(np.float32),
                                (128, VL)))
        in_maps.append(m)

    res = run_bass_kernel_spmd(
        nc, in_maps, core_ids=list(range(NCORES)),
        trace=TRACE, trace_cores=list(range(NCORES)) if TRACE else None,
        **TRACE_KWARGS,
    )
    LAST_RESULTS = res

    preds = np.zeros((B, TDF, V), np.float32)
    for c in range(NCORES):
        preds[:, :, c * VL:(c + 1) * VL] = res.results[c]["out"].transpose(1, 0, 2)
    return preds, caps, dl, sort_ind


# revision 13
# speedup vs baseline: 1.5693x; 1.4067x over previous
"""Trainium2 Bass kernel for nn_DecoderLSTM (ragged LSTM decoder + vocab projection).

Strategy (8 NeuronCores, SPMD):
  - Host: stable-sort batch by descending caption length, gather embeddings for
    the *active* (b, t) pairs only (lengths sorted desc => active set at step t
    is a prefix of the batch), pre-transpose/pack all matmul operands, cast to
    bf16.
  - Device (identical program on all 8 cores; per-core data = lin_W vocab
    shard of 4000 rows):
      * LSTM recurrence over Td steps, full batch replicated on every core.
        gates = [x; h] @ [W_ih; W_hh].T as 8 accumulating K=128 matmuls with
        the (bf16) weights as the moving operand, into one PSUM region.
      * h kept in transposed packed layout HT [128, 4, 64+NA] (bf16) via PE
        transposes, so it directly serves as the stationary operand of both
        the next-step gates matmul and the prediction matmul.
      * Prediction matmul over packed active rows in M=128 chunks (full PE
        stationary utilization), N = 4000 local vocab, interleaved with the
        recurrence as h columns become available.
      * Inactive (b, t) outputs zero-filled by DMA from a zero tile.
  - Host: concatenate the 8 vocab shards.

kernel() accepts the full unsharded inputs and returns
(predictions, caps_sorted, decode_lengths, sort_ind) exactly like reference().
"""

import math

import numpy as np
import ml_dtypes

import concourse.bass as bass
import concourse.mybir as mybir
import concourse.tile as tile
from concourse import bacc
from concourse.bass_utils import run_bass_kernel_spmd
from concourse.masks import make_identity

B, T, E, H, V = 64, 52, 512, 512, 32000
NCORES = 8
VL = V // NCORES          # 4000 local vocab rows per core
TDF = T - 1               # 51 output timesteps
G = 4 * H                 # 2048 gate columns
KE = E // 128             # 4 K-chunks for the x side
KH = H // 128             # 4 K-chunks for the h side
NT = 512                  # matmul moving-operand chunk (one PSUM bank of fp32)
PN = 500                  # prediction matmul N-chunk (4000 = 8 * 500)

F32 = mybir.dt.float32
BF16 = mybir.dt.bfloat16
AF = mybir.ActivationFunctionType

# test harness hooks
TRACE = False
TRACE_KWARGS = {}
LAST_RESULTS = None


def _build(nc, Td, bs, off, na, has_linb):
    """Emit the Tile program. bs[t] = #active batch items at step t (prefixes,
    non-increasing); off[t] = sum(bs[:t]); na = off[Td] total active pairs."""
    xt_d = nc.dram_tensor("xt", [128, KE, na], BF16, kind="ExternalInput")
    ht0_d = nc.dram_tensor("ht0", [128, KH, B], BF16, kind="ExternalInput")
    wx_d = nc.dram_tensor("wx", [128, KE, G], BF16, kind="ExternalInput")
    wh_d = nc.dram_tensor("wh", [128, KH, G], BF16, kind="ExternalInput")
    lw_d = nc.dram_tensor("lw", [128, KH, VL], BF16, kind="ExternalInput")
    bias_d = nc.dram_tensor("bias", [B, G], F32, kind="ExternalInput")
    if has_linb:
        linb_d = nc.dram_tensor("linb", [128, VL], F32, kind="ExternalInput")
    # packed active rows only: chunk m -> rows [128m, 128m+mw) contiguous, so
    # every output DMA is one big contiguous descriptor across all 16 SDMA
    # engines; the host scatters rows into the zero-initialized full output
    out_d = nc.dram_tensor("out", [na, VL], F32, kind="ExternalOutput")

    n_chunks = math.ceil(na / 128)

    with tile.TileContext(nc) as tc:
        with (
            tc.tile_pool(name="const", bufs=1) as const,
            tc.tile_pool(name="gpsum", bufs=4, space="PSUM") as gpsum,
            tc.tile_pool(name="ppsum", bufs=2, space="PSUM") as ppsum,
            tc.tile_pool(name="tpsum", bufs=2, space="PSUM") as tpsum,
            tc.tile_pool(name="work", bufs=2) as work,
            tc.tile_pool(name="pwork", bufs=3) as pwork,
        ):
            WX = const.tile([128, KE, G], BF16, tag="WX")
            nc.sync.dma_start(WX[:], wx_d[:, :, :])
            WH = const.tile([128, KH, G], BF16, tag="WH")
            nc.sync.dma_start(WH[:], wh_d[:, :, :])
            LW = const.tile([128, KH, VL], BF16, tag="LW")
            nc.sync.dma_start(LW[:], lw_d[:, :, :])
            XT = const.tile([128, KE, na], BF16, tag="XT")
            nc.sync.dma_start(XT[:], xt_d[:, :, :])
            BIAS = const.tile([B, G], F32, tag="BIAS")
            nc.sync.dma_start(BIAS[:], bias_d[:, :])
            # h-transposed history: cols 0:B = h0 (image features), block t at
            # B+off[t] (width bs[t])
            HT = const.tile([128, KH, B + na], BF16, tag="HT")
            nc.sync.dma_start(HT[:, :, 0:B], ht0_d[:, :, :])
            if has_linb:
                LB = const.tile([128, VL], F32, tag="LB")
                nc.sync.dma_start(LB[:], linb_d[:, :])
            CST = const.tile([B, H], F32, tag="CST")
            nc.vector.memset(CST[:], 0.0)
            IDN = const.tile([B, B], BF16, tag="IDN")
            make_identity(nc, IDN[:])

            def emit_pred_chunk(m):
                base = m * 128
                mw = min(128, na - base)
                pred = pwork.tile([128, VL], F32, tag="pred")
                for n in range(VL // PN):
                    pp = ppsum.tile([128, PN], F32, tag="pp")
                    for k in range(KH):
                        nc.tensor.matmul(
                            pp[:mw, :],
                            HT[:, k, B + base : B + base + mw],
                            LW[:, k, n * PN : (n + 1) * PN],
                            start=(k == 0),
                            stop=(k == KH - 1),
                        )
                    if has_linb:
                        nc.vector.tensor_add(
                            pred[:mw, n * PN : (n + 1) * PN], pp[:mw, :],
                            LB[:mw, n * PN : (n + 1) * PN],
                        )
                    else:
                        nc.vector.tensor_copy(
                            pred[:mw, n * PN : (n + 1) * PN], pp[:mw, :]
                        )
                nc.sync.dma_start(out_d[base : base + mw, :], pred[:mw, :])

            pred_done = 0
            for t in range(Td):
                bt = bs[t]
                hcol = 0 if t == 0 else B + int(off[t - 1])
                xcol = int(off[t])
                # gate layout (host-permuted): [g | i | f | o]; per-512-column
                # PSUM tiles so elementwise pipelines with the matmul stream
                acts = []
                for n in range(G // NT):
                    ps = gpsum.tile([B, NT], F32, tag="gps")
                    for k in range(KE):
                        nc.tensor.matmul(
                            ps[:bt, :],
                            XT[:, k, xcol : xcol + bt],
                            WX[:, k, n * NT : (n + 1) * NT],
                            start=(k == 0),
                            stop=False,
                        )
                    for k in range(KH):
                        nc.tensor.matmul(
                            ps[:bt, :],
                            HT[:, k, hcol : hcol + bt],
                            WH[:, k, n * NT : (n + 1) * NT],
                            start=False,
                            stop=(k == KH - 1),
                        )
                    pre = work.tile([B, NT], F32, tag=f"pre{n}")
                    nc.vector.tensor_add(
                        pre[:bt, :], ps[:bt, :], BIAS[:bt, n * NT : (n + 1) * NT]
                    )
                    av = work.tile([B, NT], F32, tag=f"act{n}")
                    nc.scalar.activation(
                        av[:bt, :], pre[:bt, :], AF.Tanh if n == 0 else AF.Sigmoid
                    )
                    acts.append(av)
                gv, iv, fv, ov = acts
                ig = work.tile([B, H], F32, tag="ig")
                nc.vector.tensor_mul(ig[:bt, :], iv[:bt, :], gv[:bt, :])
                nc.vector.tensor_mul(CST[:bt, :], fv[:bt, :], CST[:bt, :])
                nc.vector.tensor_add(CST[:bt, :], CST[:bt, :], ig[:bt, :])
                tch = work.tile([B, H], F32, tag="tch")
                nc.scalar.activation(tch[:bt, :], CST[:bt, :], AF.Tanh)
                hbf = work.tile([B, H], BF16, tag="hbf")
                nc.vector.tensor_mul(hbf[:bt, :], ov[:bt, :], tch[:bt, :])
                ocol = B + int(off[t])
                for k in range(KH):
                    tp = tpsum.tile([128, B], BF16, tag="tp")
                    nc.tensor.transpose(
                        tp[:, :bt], hbf[:bt, k * 128 : (k + 1) * 128], IDN[:bt, :bt]
                    )
                    nc.scalar.copy(HT[:, k, ocol : ocol + bt], tp[:, :bt])
                avail = int(off[t]) + bt
                while pred_done < n_chunks and (pred_done + 1) * 128 <= avail:
                    emit_pred_chunk(pred_done)
                    pred_done += 1
            while pred_done < n_chunks:
                emit_pred_chunk(pred_done)
                pred_done += 1
    return nc


def _kpack(m):
    """[D, C] (D % 128 == 0) -> [128, D//128, C] K-chunked layout."""
    d, c = m.shape
    return np.ascontiguousarray(
        m.reshape(d // 128, 128, c).transpose(1, 0, 2)
    )


def kernel(image_features, encoded_captions, caption_lengths,
           emb_W, W_ih, W_hh, b_ih, b_hh, lin_W, lin_b):
    global LAST_RESULTS
    bf = ml_dtypes.bfloat16
    img = np.asarray(image_features, np.float32)
    caps_in = np.asarray(encoded_captions)
    cl = np.asarray(caption_lengths)
    embW = np.asarray(emb_W, np.float32)
    Wih = np.asarray(W_ih, np.float32)
    Whh = np.asarray(W_hh, np.float32)
    bih = np.asarray(b_ih, np.float32)
    bhh = np.asarray(b_hh, np.float32)
    linW = np.asarray(lin_W, np.float32)
    linb = np.asarray(lin_b, np.float32)

    lengths = cl[:, 0]
    sort_ind = np.argsort(-lengths, kind="stable")
    caps = caps_in[sort_ind]
    dl = lengths[sort_ind] - 1
    img_s = img[sort_ind]

    Td = int(dl.max())
    bs = [int((dl > t).sum()) for t in range(Td)]
    off = np.concatenate([[0], np.cumsum(bs)]).astype(np.int64)
    na = int(off[Td])

    xp = np.empty((na, E), np.float32)
    for t in range(Td):
        xp[off[t]:off[t + 1]] = embW[caps[: bs[t], t]]

    # gate order [g, i, f, o] (torch rows are i, f, g, o)
    perm = np.concatenate([
        np.arange(2 * H, 3 * H), np.arange(0, H),
        np.arange(H, 2 * H), np.arange(3 * H, 4 * H),
    ])
    xt = _kpack(np.ascontiguousarray(xp.T).astype(bf))
    ht0 = _kpack(np.ascontiguousarray(img_s.T).astype(bf))
    wx = _kpack(np.ascontiguousarray(Wih[perm].T).astype(bf))
    wh = _kpack(np.ascontiguousarray(Whh[perm].T).astype(bf))
    bias = np.ascontiguousarray(
        np.broadcast_to((bih + bhh)[perm].astype(np.float32), (B, G))
    )
    has_linb = bool(np.any(linb != 0.0))

    nc = bacc.Bacc("TRN2", target_bir_lowering=False, debug=False,
                   num_devices=NCORES)
    _build(nc, Td, bs, off, na, has_linb)
    nc.finalize()

    in_maps = []
    for c in range(NCORES):
        lw = _kpack(np.ascontiguousarray(linW[c * VL:(c + 1) * VL].T).astype(bf))
        m = dict(xt=xt, ht0=ht0, wx=wx, wh=wh, lw=lw, bias=bias)
        if has_linb:
            m["linb"] = np.ascontiguousarray(
                np.broadcast_to(linb[c * VL:(c + 1) * VL].astype(np.float32),
                                (128, VL)))
        in_maps.append(m)

    res = run_bass_kernel_spmd(
        nc, in_maps, core_ids=list(range(NCORES)),
        trace=TRACE, trace_cores=list(range(NCORES)) if TRACE else None,
        **TRACE_KWARGS,
    )
    LAST_RESULTS = res

    preds = np.zeros((B, TDF, V), np.float32)
    for c in range(NCORES):
        o = res.results[c]["out"]
        sh = slice(c * VL, (c + 1) * VL)
        for t in range(Td):
            preds[: bs[t], t, sh] = o[off[t]:off[t + 1]]
    return preds, caps, dl, sort_ind
